# revision 1
# baseline (speedup 1.0000x reference)
"""Trainium2 Bass kernel for nn_ContextAddition (ragged sequence insertion).

Math: for each row b with first-EOT position e = argmin{p: tok[b,p]==EOT} and
shift = 16 if dynamic_bools[b] else 8, the reference output reduces to a pure
row-gather from an extended embedding table T = [token_embedding; da; ca]:

    out[b,p] = T[ tok[b,p] ]            if p <  e
             = T[ VOCAB + (p - e) ]     if e <= p < e + shift   (da rows then ca rows)
             = T[ tok[b, p - shift] ]   if p >= e + shift

(The da insertion applies to all rows; the ca insertion only to dynamic rows,
and since da precedes ca in T, VOCAB + (p - e) indexes both uniformly.)

So the kernel computes an int32 index map [B,77] on-device with vector ops,
then does one big indirect-DMA row gather (3072 B/row) from DRAM, staged
through SBUF, written densely to the output. Pure data parallel over 8 cores
(256 batch rows each); the embedding table is replicated.

Device-input layout: tokens/dynamic_bools/position-iota are packed into one
f32 "meta" array [B, 2*SEQ+1] host-side so the whole per-tile index
computation hangs off a single input DMA (all values < 2^24, f32-exact).
"""

import sys

import numpy as np

from concourse import bacc, bass, mybir
import concourse.tile as tile
from concourse.bass_utils import run_bass_kernel_spmd


def _ensure_profiling_hooks():
    """Make NTFF tracing under axon non-fatal / functional if BASS_TRACE is
    set by the caller: register the antenv.axon_hooks shim when the real
    module is absent, and make artifact upload failures non-fatal."""
    try:
        import antenv.axon_hooks  # noqa: F401
    except ImportError:
        try:
            import contextlib as _cl
            import types as _t

            import antenv
            from trn_agent_boot.trn_boot import _ntff_profile_via_ctypes

            hook = _ntff_profile_via_ctypes("/opt/axon/libaxon_pjrt.so")

            if hook is not None:
                _raw = hook

                @_cl.contextmanager
                def _safe(output_dir, device_ids):
                    # transient axon profiler failures (e.g. stop rc=-1)
                    # degrade to "no trace" instead of crashing the run
                    try:
                        cm = _raw(output_dir, device_ids)
                        cm.__enter__()
                    except Exception:
                        yield
                        return
                    try:
                        yield
                    finally:
                        try:
                            cm.__exit__(None, None, None)
                        except Exception:
                            pass

                hook = _safe

            mod = _t.ModuleType("antenv.axon_hooks")
            mod._hook = hook
            mod.set_axon_ntff_profile_hook = lambda h: setattr(mod, "_hook", h)
            mod.get_axon_ntff_profile_hook = lambda: mod._hook
            sys.modules["antenv.axon_hooks"] = mod
            antenv.axon_hooks = mod
        except Exception:
            pass
    from concourse import bass_utils as _bu

    if not getattr(_bu.upload_artifacts, "_safe_wrapped", False):
        _orig = _bu.upload_artifacts

        def _safe_upload(tmpdir):
            try:
                return _orig(tmpdir)
            except Exception:
                return f"file://{tmpdir}"

        _safe_upload._safe_wrapped = True
        _bu.upload_artifacts = _safe_upload

B, SEQ, DIM = 2048, 77, 768
VOCAB, EOT = 49408, 49407
INS = 16                       # appended rows: 8 da + 8 ca
TBL = VOCAB + INS
NCORES = 8
BPC = B // NCORES              # 256 batch rows per core
P = 128
NT = BPC // P                  # 2 partition tiles per core
SC = 11                        # seq chunk: 77 = 7 * 11
NCH = SEQ // SC
MW = 2 * SEQ + 1               # meta width: [tokens | dyn | pos]
TABLE_DT = "f16"               # "f32": exact; "f16": half-size table, cast-on-write (rel err ~2e-4)
WRITE_MODE = "swdge"           # "swdge": cast during write DMA; "dve": DVE cast + HWDGE write
GP_BUFS = 4                    # gather pool depth
DMA_SCRATCH = 16384            # SWDGE descriptor-ring carveout bytes
# Alternating 6/5 position chunks: writes enter the SWDGE ring every ~6
# gathers, interleaving sequential write packets between latency-bound
# random-read gather packets (measured faster and more noise-robust than
# 7x11 uniform chunks).
CHUNKS = [6, 5] * 7

f32 = mybir.dt.float32
i32 = mybir.dt.int32
Alu = mybir.AluOpType


def _build() -> bass.Bass:
    global TABLE_DT, WRITE_MODE, GP_BUFS, CHUNKS, DMA_SCRATCH
    chunks = CHUNKS if CHUNKS is not None else [SC] * NCH
    assert sum(chunks) == SEQ
    tdt = f32 if TABLE_DT == "f32" else mybir.dt.float16
    nc = bacc.Bacc("TRN2", dynamic_dma_scratch_size=DMA_SCRATCH)
    meta_ext = nc.declare_dram_parameter("meta", [BPC, MW], f32, isOutput=False)
    table_ext = nc.declare_dram_parameter("table", [TBL, DIM], tdt, isOutput=False)
    out_ext = nc.declare_dram_parameter("out", [BPC, SEQ * DIM], f32, isOutput=True)

    with tile.TileContext(nc) as tc:
        with (
            tc.tile_pool(name="small", bufs=2) as sp,
            tc.tile_pool(name="gath", bufs=GP_BUFS) as gp,
            tc.tile_pool(name="cast", bufs=3) as hp,
        ):
            for t in range(NT):
                rows = slice(t * P, (t + 1) * P)

                meta = sp.tile([P, MW], f32, tag="meta")
                nc.sync.dma_start(out=meta[:], in_=meta_ext[rows, :])
                tok = meta[:, 0:SEQ]
                dyn = meta[:, SEQ : SEQ + 1]
                pos = meta[:, SEQ + 1 : SEQ + 1 + SEQ]

                # e[b] = sum_p p * (tok == EOT)  (exactly one EOT per row)
                iseq = sp.tile([P, SEQ], f32, tag="iseq")
                nc.vector.tensor_scalar(
                    out=iseq[:], in0=tok, scalar1=float(EOT), scalar2=None,
                    op0=Alu.is_equal,
                )
                pe = sp.tile([P, SEQ], f32, tag="pe")
                nc.vector.tensor_tensor(out=pe[:], in0=iseq[:], in1=pos, op=Alu.mult)
                e = sp.tile([P, 1], f32, tag="e")
                nc.vector.tensor_reduce(
                    out=e[:], in_=pe[:], axis=mybir.AxisListType.X, op=Alu.add
                )

                # eth[b] = e + 8 + 8*dyn
                sh = sp.tile([P, 1], f32, tag="sh")
                nc.vector.tensor_scalar(
                    out=sh[:], in0=dyn, scalar1=8.0, scalar2=8.0,
                    op0=Alu.mult, op1=Alu.add,
                )
                eth = sp.tile([P, 1], f32, tag="eth")
                nc.vector.tensor_tensor(out=eth[:], in0=sh[:], in1=e[:], op=Alu.add)

                # mid = pos - e + VOCAB   (index into the da/ca rows)
                mid = sp.tile([P, SEQ], f32, tag="mid")
                nc.vector.tensor_scalar(
                    out=mid[:], in0=pos, scalar1=e[:], scalar2=float(VOCAB),
                    op0=Alu.subtract, op1=Alu.add,
                )

                # masks must be integer-typed for CopyPredicated on HW
                m1 = sp.tile([P, SEQ], i32, tag="m1")   # p < e
                nc.vector.tensor_scalar(
                    out=m1[:], in0=pos, scalar1=e[:], scalar2=None, op0=Alu.is_lt
                )
                m2 = sp.tile([P, SEQ], i32, tag="m2")   # p < e + shift
                nc.vector.tensor_scalar(
                    out=m2[:], in0=pos, scalar1=eth[:], scalar2=None, op0=Alu.is_lt
                )

                # tok shifted right by 8 and by 16 (cols < shift never selected)
                tm8 = sp.tile([P, SEQ], f32, tag="tm8")
                nc.vector.tensor_copy(out=tm8[:, 8:SEQ], in_=meta[:, 0 : SEQ - 8])
                nc.vector.tensor_copy(out=tm8[:, 0:8], in_=meta[:, 0:8])
                tm16 = sp.tile([P, SEQ], f32, tag="tm16")
                nc.vector.tensor_copy(out=tm16[:, 16:SEQ], in_=meta[:, 0 : SEQ - 16])
                nc.vector.tensor_copy(out=tm16[:, 0:16], in_=meta[:, 0:16])

                # sel = tm8 + dyn * (tm16 - tm8); overlay mid, then pre-EOT tokens
                dd = sp.tile([P, SEQ], f32, tag="dd")
                nc.vector.tensor_tensor(out=dd[:], in0=tm16[:], in1=tm8[:], op=Alu.subtract)
                ddm = sp.tile([P, SEQ], f32, tag="ddm")
                nc.vector.tensor_scalar(
                    out=ddm[:], in0=dd[:], scalar1=dyn, scalar2=None, op0=Alu.mult
                )
                sel = sp.tile([P, SEQ], f32, tag="sel")
                nc.vector.tensor_tensor(out=sel[:], in0=tm8[:], in1=ddm[:], op=Alu.add)
                nc.vector.copy_predicated(out=sel[:], mask=m2[:], data=mid[:])
                nc.vector.copy_predicated(out=sel[:], mask=m1[:], data=tok)

                idx = sp.tile([P, SEQ], i32, tag="idx")
                nc.vector.tensor_copy(out=idx[:], in_=sel[:])

                s0 = 0
                for c, cl in enumerate(chunks):
                    # one indirect DMA per position, [128,1] offsets (one
                    # index per partition): the HW DGE emits one descriptor
                    # per partition, consuming exactly one offset element
                    # each (multi-index-per-partition forms misbehave on HW)
                    g = gp.tile([P, cl, DIM], tdt, tag="g")
                    for j in range(cl):
                        nc.gpsimd.indirect_dma_start(
                            out=g[:, j, :],
                            out_offset=None,
                            in_=table_ext[:],
                            in_offset=bass.IndirectOffsetOnAxis(
                                ap=idx[:, s0 + j : s0 + j + 1], axis=0
                            ),
                        )
                    if TABLE_DT == "f32":
                        nc.sync.dma_start(
                            out=out_ext[rows, s0 * DIM : (s0 + cl) * DIM],
                            in_=g[:, :, :],
                        )
                    elif WRITE_MODE == "swdge":
                        # dtype cast during DMA requires SWDGE (gpsimd)
                        nc.gpsimd.dma_start(
                            out=out_ext[rows, s0 * DIM : (s0 + cl) * DIM],
                            in_=g[:, :, :],
                        )
                    else:
                        # DVE casts fp16->f32 in SBUF; HWDGE writes f32.
                        # Keeps the SWDGE ring gather-only and shortens the
                        # gather-slot reuse chain (waits on the cast, not
                        # the write DMA).
                        h = hp.tile([P, cl, DIM], f32, tag="h")
                        nc.vector.tensor_copy(out=h[:, :, :], in_=g[:, :, :])
                        nc.sync.dma_start(
                            out=out_ext[rows, s0 * DIM : (s0 + cl) * DIM],
                            in_=h[:, :, :],
                        )
                    s0 += cl
    nc.finalize()
    return nc


_cache: dict = {}


def _pack_meta(tokens_i32: np.ndarray, dyn_i32: np.ndarray) -> np.ndarray:
    meta = np.empty((B, MW), dtype=np.float32)
    meta[:, 0:SEQ] = tokens_i32
    meta[:, SEQ] = dyn_i32
    meta[:, SEQ + 1 :] = np.arange(SEQ, dtype=np.float32)[None, :]
    return meta


def kernel(**inputs) -> np.ndarray:
    _ensure_profiling_hooks()
    tokens = np.asarray(inputs["tokenized_text"]).astype(np.int32)
    dyn = np.asarray(inputs["dynamic_bools"]).astype(np.int32)
    emb = np.asarray(inputs["token_embedding"], dtype=np.float32)
    da = np.asarray(inputs["da_vectors"], dtype=np.float32)
    ca = np.asarray(inputs["ca_vectors"], dtype=np.float32)
    table = np.ascontiguousarray(np.concatenate([emb, da, ca], axis=0))
    if TABLE_DT == "f16":
        table = table.astype(np.float16)
    meta = _pack_meta(tokens, dyn)

    if "nc" not in _cache:
        _cache["nc"] = _build()
    nc = _cache["nc"]

    in_maps = []
    for i in range(NCORES):
        rows = slice(i * BPC, (i + 1) * BPC)
        in_maps.append({"meta": meta[rows], "table": table})
    res = run_bass_kernel_spmd(nc, in_maps, core_ids=list(range(NCORES)))
    _cache["last_results"] = res
    out = np.concatenate(
        [res.results[i]["out"].reshape(BPC, SEQ, DIM) for i in range(NCORES)],
        axis=0,
    )
    return out



# revision 5
# speedup vs baseline: 1.5175x; 1.5175x over previous
"""Trainium2 Bass kernel for nn_ContextAddition (ragged sequence insertion).

Math: for each row b with first-EOT position e = argmin{p: tok[b,p]==EOT} and
shift = 16 if dynamic_bools[b] else 8, the reference output reduces to a pure
row-gather from an extended embedding table T = [token_embedding; da; ca]:

    out[b,p] = T[ tok[b,p] ]            if p <  e
             = T[ VOCAB + (p - e) ]     if e <= p < e + shift   (da rows then ca rows)
             = T[ tok[b, p - shift] ]   if p >= e + shift

(The da insertion applies to all rows; the ca insertion only to dynamic rows,
and since da precedes ca in T, VOCAB + (p - e) indexes both uniformly.)

So the kernel computes an int32 index map [B,77] on-device with vector ops,
then does one big indirect-DMA row gather (3072 B/row) from DRAM, staged
through SBUF, written densely to the output. Pure data parallel over 8 cores
(256 batch rows each); the embedding table is replicated.

Device-input layout: tokens/dynamic_bools/position-iota are packed into one
f32 "meta" array [B, 2*SEQ+1] host-side so the whole per-tile index
computation hangs off a single input DMA (all values < 2^24, f32-exact).
"""

import sys

import numpy as np

from concourse import bacc, bass, mybir
import concourse.tile as tile
from concourse.bass_utils import run_bass_kernel_spmd


def _ensure_profiling_hooks():
    """Make NTFF tracing under axon non-fatal / functional if BASS_TRACE is
    set by the caller: register the antenv.axon_hooks shim when the real
    module is absent, and make artifact upload failures non-fatal."""
    try:
        import antenv.axon_hooks  # noqa: F401
    except ImportError:
        try:
            import contextlib as _cl
            import types as _t

            import antenv
            from trn_agent_boot.trn_boot import _ntff_profile_via_ctypes

            hook = _ntff_profile_via_ctypes("/opt/axon/libaxon_pjrt.so")

            if hook is not None:
                _raw = hook

                @_cl.contextmanager
                def _safe(output_dir, device_ids):
                    # transient axon profiler failures (e.g. stop rc=-1)
                    # degrade to "no trace" instead of crashing the run
                    try:
                        cm = _raw(output_dir, device_ids)
                        cm.__enter__()
                    except Exception:
                        yield
                        return
                    try:
                        yield
                    finally:
                        try:
                            cm.__exit__(None, None, None)
                        except Exception:
                            pass

                hook = _safe

            mod = _t.ModuleType("antenv.axon_hooks")
            mod._hook = hook
            mod.set_axon_ntff_profile_hook = lambda h: setattr(mod, "_hook", h)
            mod.get_axon_ntff_profile_hook = lambda: mod._hook
            sys.modules["antenv.axon_hooks"] = mod
            antenv.axon_hooks = mod
        except Exception:
            pass
    from concourse import bass_utils as _bu

    if not getattr(_bu.upload_artifacts, "_safe_wrapped", False):
        _orig = _bu.upload_artifacts

        def _safe_upload(tmpdir):
            try:
                return _orig(tmpdir)
            except Exception:
                return f"file://{tmpdir}"

        _safe_upload._safe_wrapped = True
        _bu.upload_artifacts = _safe_upload

B, SEQ, DIM = 2048, 77, 768
VOCAB, EOT = 49408, 49407
INS = 16                       # appended rows: 8 da + 8 ca
TBL = VOCAB + INS
NCORES = 8
BPC = B // NCORES              # 256 batch rows per core
P = 128
NT = BPC // P                  # 2 partition tiles per core
SC = 11                        # seq chunk: 77 = 7 * 11
NCH = SEQ // SC
MW = 2 * SEQ + 1               # meta width: [tokens | dyn | pos]
TABLE_DT = "f16"               # "f32": exact; "f16": half-size table (rel err ~2e-4)
OUT_DT = "f16"                 # "f16": write output f16, upcast host-side (lossless
                               # vs the f16 table); halves write-side HBM traffic
GP_BUFS = 4                    # gather pool depth
DMA_SCRATCH = 16384            # SWDGE descriptor-ring carveout bytes
# Alternating 6/5 position chunks: writes enter the SWDGE ring every ~6
# gathers, interleaving sequential write packets between latency-bound
# random-read gather packets (measured faster and more noise-robust than
# 7x11 uniform chunks).
CHUNKS = [6, 5] * 7

f32 = mybir.dt.float32
i32 = mybir.dt.int32
Alu = mybir.AluOpType


def _build() -> bass.Bass:
    global TABLE_DT, OUT_DT, GP_BUFS, CHUNKS, DMA_SCRATCH
    chunks = CHUNKS if CHUNKS is not None else [SC] * NCH
    assert sum(chunks) == SEQ
    tdt = f32 if TABLE_DT == "f32" else mybir.dt.float16
    odt = f32 if OUT_DT == "f32" else mybir.dt.float16
    nc = bacc.Bacc("TRN2", dynamic_dma_scratch_size=DMA_SCRATCH)
    meta_ext = nc.declare_dram_parameter("meta", [BPC, MW], f32, isOutput=False)
    table_ext = nc.declare_dram_parameter("table", [TBL, DIM], tdt, isOutput=False)
    out_ext = nc.declare_dram_parameter("out", [BPC, SEQ * DIM], odt, isOutput=True)

    with tile.TileContext(nc) as tc:
        with (
            tc.tile_pool(name="small", bufs=2) as sp,
            tc.tile_pool(name="gath", bufs=GP_BUFS) as gp,
            tc.tile_pool(name="cast", bufs=3) as hp,
        ):
            for t in range(NT):
                rows = slice(t * P, (t + 1) * P)

                meta = sp.tile([P, MW], f32, tag="meta")
                nc.sync.dma_start(out=meta[:], in_=meta_ext[rows, :])
                tok = meta[:, 0:SEQ]
                dyn = meta[:, SEQ : SEQ + 1]
                pos = meta[:, SEQ + 1 : SEQ + 1 + SEQ]

                # e[b] = sum_p p * (tok == EOT)  (exactly one EOT per row)
                iseq = sp.tile([P, SEQ], f32, tag="iseq")
                nc.vector.tensor_scalar(
                    out=iseq[:], in0=tok, scalar1=float(EOT), scalar2=None,
                    op0=Alu.is_equal,
                )
                pe = sp.tile([P, SEQ], f32, tag="pe")
                nc.vector.tensor_tensor(out=pe[:], in0=iseq[:], in1=pos, op=Alu.mult)
                e = sp.tile([P, 1], f32, tag="e")
                nc.vector.tensor_reduce(
                    out=e[:], in_=pe[:], axis=mybir.AxisListType.X, op=Alu.add
                )

                # eth[b] = e + 8 + 8*dyn
                sh = sp.tile([P, 1], f32, tag="sh")
                nc.vector.tensor_scalar(
                    out=sh[:], in0=dyn, scalar1=8.0, scalar2=8.0,
                    op0=Alu.mult, op1=Alu.add,
                )
                eth = sp.tile([P, 1], f32, tag="eth")
                nc.vector.tensor_tensor(out=eth[:], in0=sh[:], in1=e[:], op=Alu.add)

                # mid = pos - e + VOCAB   (index into the da/ca rows)
                mid = sp.tile([P, SEQ], f32, tag="mid")
                nc.vector.tensor_scalar(
                    out=mid[:], in0=pos, scalar1=e[:], scalar2=float(VOCAB),
                    op0=Alu.subtract, op1=Alu.add,
                )

                # masks must be integer-typed for CopyPredicated on HW
                m1 = sp.tile([P, SEQ], i32, tag="m1")   # p < e
                nc.vector.tensor_scalar(
                    out=m1[:], in0=pos, scalar1=e[:], scalar2=None, op0=Alu.is_lt
                )
                m2 = sp.tile([P, SEQ], i32, tag="m2")   # p < e + shift
                nc.vector.tensor_scalar(
                    out=m2[:], in0=pos, scalar1=eth[:], scalar2=None, op0=Alu.is_lt
                )

                # tok shifted right by 8 and by 16 (cols < shift never selected)
                tm8 = sp.tile([P, SEQ], f32, tag="tm8")
                nc.vector.tensor_copy(out=tm8[:, 8:SEQ], in_=meta[:, 0 : SEQ - 8])
                nc.vector.tensor_copy(out=tm8[:, 0:8], in_=meta[:, 0:8])
                tm16 = sp.tile([P, SEQ], f32, tag="tm16")
                nc.vector.tensor_copy(out=tm16[:, 16:SEQ], in_=meta[:, 0 : SEQ - 16])
                nc.vector.tensor_copy(out=tm16[:, 0:16], in_=meta[:, 0:16])

                # sel = tm8 + dyn * (tm16 - tm8); overlay mid, then pre-EOT tokens
                dd = sp.tile([P, SEQ], f32, tag="dd")
                nc.vector.tensor_tensor(out=dd[:], in0=tm16[:], in1=tm8[:], op=Alu.subtract)
                ddm = sp.tile([P, SEQ], f32, tag="ddm")
                nc.vector.tensor_scalar(
                    out=ddm[:], in0=dd[:], scalar1=dyn, scalar2=None, op0=Alu.mult
                )
                sel = sp.tile([P, SEQ], f32, tag="sel")
                nc.vector.tensor_tensor(out=sel[:], in0=tm8[:], in1=ddm[:], op=Alu.add)
                nc.vector.copy_predicated(out=sel[:], mask=m2[:], data=mid[:])
                nc.vector.copy_predicated(out=sel[:], mask=m1[:], data=tok)

                idx = sp.tile([P, SEQ], i32, tag="idx")
                nc.vector.tensor_copy(out=idx[:], in_=sel[:])

                s0 = 0
                for c, cl in enumerate(chunks):
                    # one indirect DMA per position, [128,1] offsets (one
                    # index per partition): the HW DGE emits one descriptor
                    # per partition, consuming exactly one offset element
                    # each (multi-index-per-partition forms misbehave on HW)
                    g = gp.tile([P, cl, DIM], tdt, tag="g")
                    for j in range(cl):
                        nc.gpsimd.indirect_dma_start(
                            out=g[:, j, :],
                            out_offset=None,
                            in_=table_ext[:],
                            in_offset=bass.IndirectOffsetOnAxis(
                                ap=idx[:, s0 + j : s0 + j + 1], axis=0
                            ),
                        )
                    if TABLE_DT == OUT_DT:
                        # same dtype: plain HWDGE write, no cast anywhere
                        nc.sync.dma_start(
                            out=out_ext[rows, s0 * DIM : (s0 + cl) * DIM],
                            in_=g[:, :, :],
                        )
                    else:
                        # dtype cast during DMA requires SWDGE (gpsimd)
                        nc.gpsimd.dma_start(
                            out=out_ext[rows, s0 * DIM : (s0 + cl) * DIM],
                            in_=g[:, :, :],
                        )
                    s0 += cl
    nc.finalize()
    return nc


_cache: dict = {}


def _pack_meta(tokens_i32: np.ndarray, dyn_i32: np.ndarray) -> np.ndarray:
    meta = np.empty((B, MW), dtype=np.float32)
    meta[:, 0:SEQ] = tokens_i32
    meta[:, SEQ] = dyn_i32
    meta[:, SEQ + 1 :] = np.arange(SEQ, dtype=np.float32)[None, :]
    return meta


def kernel(**inputs) -> np.ndarray:
    _ensure_profiling_hooks()
    tokens = np.asarray(inputs["tokenized_text"]).astype(np.int32)
    dyn = np.asarray(inputs["dynamic_bools"]).astype(np.int32)
    emb = np.asarray(inputs["token_embedding"], dtype=np.float32)
    da = np.asarray(inputs["da_vectors"], dtype=np.float32)
    ca = np.asarray(inputs["ca_vectors"], dtype=np.float32)
    table = np.ascontiguousarray(np.concatenate([emb, da, ca], axis=0))
    if TABLE_DT == "f16":
        table = table.astype(np.float16)
    meta = _pack_meta(tokens, dyn)

    if "nc" not in _cache:
        _cache["nc"] = _build()
    nc = _cache["nc"]

    in_maps = []
    for i in range(NCORES):
        rows = slice(i * BPC, (i + 1) * BPC)
        in_maps.append({"meta": meta[rows], "table": table})
    res = run_bass_kernel_spmd(nc, in_maps, core_ids=list(range(NCORES)))
    _cache["last_results"] = res
    out = np.concatenate(
        [
            res.results[i]["out"].reshape(BPC, SEQ, DIM).astype(np.float32)
            for i in range(NCORES)
        ],
        axis=0,
    )
    return out



# revision 6
# speedup vs baseline: 1.5189x; 1.0009x over previous
"""Trainium2 Bass kernel for nn_ContextAddition (ragged sequence insertion).

Math: for each row b with first-EOT position e = argmin{p: tok[b,p]==EOT} and
shift = 16 if dynamic_bools[b] else 8, the reference output reduces to a pure
row-gather from an extended embedding table T = [token_embedding; da; ca]:

    out[b,p] = T[ tok[b,p] ]            if p <  e
             = T[ VOCAB + (p - e) ]     if e <= p < e + shift   (da rows then ca rows)
             = T[ tok[b, p - shift] ]   if p >= e + shift

(The da insertion applies to all rows; the ca insertion only to dynamic rows,
and since da precedes ca in T, VOCAB + (p - e) indexes both uniformly.)

So the kernel computes an int32 index map [B,77] on-device with vector ops,
then does one big indirect-DMA row gather (3072 B/row) from DRAM, staged
through SBUF, written densely to the output. Pure data parallel over 8 cores
(256 batch rows each); the embedding table is replicated.

Device-input layout: tokens/dynamic_bools/position-iota are packed into one
f32 "meta" array [B, 2*SEQ+1] host-side so the whole per-tile index
computation hangs off a single input DMA (all values < 2^24, f32-exact).
"""

import sys

import numpy as np

from concourse import bacc, bass, mybir
import concourse.tile as tile
from concourse.bass_utils import run_bass_kernel_spmd


def _ensure_profiling_hooks():
    """Make NTFF tracing under axon non-fatal / functional if BASS_TRACE is
    set by the caller: register the antenv.axon_hooks shim when the real
    module is absent, and make artifact upload failures non-fatal."""
    try:
        import antenv.axon_hooks  # noqa: F401
    except ImportError:
        try:
            import contextlib as _cl
            import types as _t

            import antenv
            from trn_agent_boot.trn_boot import _ntff_profile_via_ctypes

            hook = _ntff_profile_via_ctypes("/opt/axon/libaxon_pjrt.so")

            if hook is not None:
                _raw = hook

                @_cl.contextmanager
                def _safe(output_dir, device_ids):
                    # transient axon profiler failures (e.g. stop rc=-1)
                    # degrade to "no trace" instead of crashing the run
                    try:
                        cm = _raw(output_dir, device_ids)
                        cm.__enter__()
                    except Exception:
                        yield
                        return
                    try:
                        yield
                    finally:
                        try:
                            cm.__exit__(None, None, None)
                        except Exception:
                            pass

                hook = _safe

            mod = _t.ModuleType("antenv.axon_hooks")
            mod._hook = hook
            mod.set_axon_ntff_profile_hook = lambda h: setattr(mod, "_hook", h)
            mod.get_axon_ntff_profile_hook = lambda: mod._hook
            sys.modules["antenv.axon_hooks"] = mod
            antenv.axon_hooks = mod
        except Exception:
            pass
    from concourse import bass_utils as _bu

    if not getattr(_bu.upload_artifacts, "_safe_wrapped", False):
        _orig = _bu.upload_artifacts

        def _safe_upload(tmpdir):
            try:
                return _orig(tmpdir)
            except Exception:
                return f"file://{tmpdir}"

        _safe_upload._safe_wrapped = True
        _bu.upload_artifacts = _safe_upload

B, SEQ, DIM = 2048, 77, 768
VOCAB, EOT = 49408, 49407
INS = 16                       # appended rows: 8 da + 8 ca
TBL = VOCAB + INS
NCORES = 8
BPC = B // NCORES              # 256 batch rows per core
P = 128
NT = BPC // P                  # 2 partition tiles per core
SC = 11                        # seq chunk: 77 = 7 * 11
NCH = SEQ // SC
MW = 2 * SEQ + 1               # meta width: [tokens | dyn | pos]
TABLE_DT = "f16"               # "f32": exact; "f16": half-size table (rel err ~2e-4)
OUT_DT = "f16"                 # "f16": write output f16, upcast host-side (lossless
                               # vs the f16 table); halves write-side HBM traffic
GP_BUFS = 8                    # gather pool depth
DMA_SCRATCH = 16384            # SWDGE descriptor-ring carveout bytes
# With f16 HWDGE writes (no SWDGE cast-writes in the ring anymore) the
# old 6/5 interleave rationale is gone; bigger chunks = fewer, larger
# writes and a deeper gather pipeline.
CHUNKS = [11] * 7

f32 = mybir.dt.float32
i32 = mybir.dt.int32
Alu = mybir.AluOpType


def _build() -> bass.Bass:
    global TABLE_DT, OUT_DT, GP_BUFS, CHUNKS, DMA_SCRATCH
    chunks = CHUNKS if CHUNKS is not None else [SC] * NCH
    assert sum(chunks) == SEQ
    tdt = f32 if TABLE_DT == "f32" else mybir.dt.float16
    odt = f32 if OUT_DT == "f32" else mybir.dt.float16
    nc = bacc.Bacc("TRN2", dynamic_dma_scratch_size=DMA_SCRATCH)
    meta_ext = nc.declare_dram_parameter("meta", [BPC, MW], f32, isOutput=False)
    table_ext = nc.declare_dram_parameter("table", [TBL, DIM], tdt, isOutput=False)
    out_ext = nc.declare_dram_parameter("out", [BPC, SEQ * DIM], odt, isOutput=True)

    with tile.TileContext(nc) as tc:
        with (
            tc.tile_pool(name="small", bufs=2) as sp,
            tc.tile_pool(name="gath", bufs=GP_BUFS) as gp,
            tc.tile_pool(name="cast", bufs=3) as hp,
        ):
            for t in range(NT):
                rows = slice(t * P, (t + 1) * P)

                meta = sp.tile([P, MW], f32, tag="meta")
                nc.sync.dma_start(out=meta[:], in_=meta_ext[rows, :])
                tok = meta[:, 0:SEQ]
                dyn = meta[:, SEQ : SEQ + 1]
                pos = meta[:, SEQ + 1 : SEQ + 1 + SEQ]

                # e[b] = sum_p p * (tok == EOT)  (exactly one EOT per row)
                iseq = sp.tile([P, SEQ], f32, tag="iseq")
                nc.vector.tensor_scalar(
                    out=iseq[:], in0=tok, scalar1=float(EOT), scalar2=None,
                    op0=Alu.is_equal,
                )
                pe = sp.tile([P, SEQ], f32, tag="pe")
                nc.vector.tensor_tensor(out=pe[:], in0=iseq[:], in1=pos, op=Alu.mult)
                e = sp.tile([P, 1], f32, tag="e")
                nc.vector.tensor_reduce(
                    out=e[:], in_=pe[:], axis=mybir.AxisListType.X, op=Alu.add
                )

                # eth[b] = e + 8 + 8*dyn
                sh = sp.tile([P, 1], f32, tag="sh")
                nc.vector.tensor_scalar(
                    out=sh[:], in0=dyn, scalar1=8.0, scalar2=8.0,
                    op0=Alu.mult, op1=Alu.add,
                )
                eth = sp.tile([P, 1], f32, tag="eth")
                nc.vector.tensor_tensor(out=eth[:], in0=sh[:], in1=e[:], op=Alu.add)

                # mid = pos - e + VOCAB   (index into the da/ca rows)
                mid = sp.tile([P, SEQ], f32, tag="mid")
                nc.vector.tensor_scalar(
                    out=mid[:], in0=pos, scalar1=e[:], scalar2=float(VOCAB),
                    op0=Alu.subtract, op1=Alu.add,
                )

                # masks must be integer-typed for CopyPredicated on HW
                m1 = sp.tile([P, SEQ], i32, tag="m1")   # p < e
                nc.vector.tensor_scalar(
                    out=m1[:], in0=pos, scalar1=e[:], scalar2=None, op0=Alu.is_lt
                )
                m2 = sp.tile([P, SEQ], i32, tag="m2")   # p < e + shift
                nc.vector.tensor_scalar(
                    out=m2[:], in0=pos, scalar1=eth[:], scalar2=None, op0=Alu.is_lt
                )

                # tok shifted right by 8 and by 16 (cols < shift never selected)
                tm8 = sp.tile([P, SEQ], f32, tag="tm8")
                nc.vector.tensor_copy(out=tm8[:, 8:SEQ], in_=meta[:, 0 : SEQ - 8])
                nc.vector.tensor_copy(out=tm8[:, 0:8], in_=meta[:, 0:8])
                tm16 = sp.tile([P, SEQ], f32, tag="tm16")
                nc.vector.tensor_copy(out=tm16[:, 16:SEQ], in_=meta[:, 0 : SEQ - 16])
                nc.vector.tensor_copy(out=tm16[:, 0:16], in_=meta[:, 0:16])

                # sel = tm8 + dyn * (tm16 - tm8); overlay mid, then pre-EOT tokens
                dd = sp.tile([P, SEQ], f32, tag="dd")
                nc.vector.tensor_tensor(out=dd[:], in0=tm16[:], in1=tm8[:], op=Alu.subtract)
                ddm = sp.tile([P, SEQ], f32, tag="ddm")
                nc.vector.tensor_scalar(
                    out=ddm[:], in0=dd[:], scalar1=dyn, scalar2=None, op0=Alu.mult
                )
                sel = sp.tile([P, SEQ], f32, tag="sel")
                nc.vector.tensor_tensor(out=sel[:], in0=tm8[:], in1=ddm[:], op=Alu.add)
                nc.vector.copy_predicated(out=sel[:], mask=m2[:], data=mid[:])
                nc.vector.copy_predicated(out=sel[:], mask=m1[:], data=tok)

                idx = sp.tile([P, SEQ], i32, tag="idx")
                nc.vector.tensor_copy(out=idx[:], in_=sel[:])

                s0 = 0
                for c, cl in enumerate(chunks):
                    # one indirect DMA per position, [128,1] offsets (one
                    # index per partition): the HW DGE emits one descriptor
                    # per partition, consuming exactly one offset element
                    # each (multi-index-per-partition forms misbehave on HW)
                    g = gp.tile([P, cl, DIM], tdt, tag="g")
                    for j in range(cl):
                        nc.gpsimd.indirect_dma_start(
                            out=g[:, j, :],
                            out_offset=None,
                            in_=table_ext[:],
                            in_offset=bass.IndirectOffsetOnAxis(
                                ap=idx[:, s0 + j : s0 + j + 1], axis=0
                            ),
                        )
                    if TABLE_DT == OUT_DT:
                        # same dtype: plain HWDGE write, no cast anywhere
                        nc.sync.dma_start(
                            out=out_ext[rows, s0 * DIM : (s0 + cl) * DIM],
                            in_=g[:, :, :],
                        )
                    else:
                        # dtype cast during DMA requires SWDGE (gpsimd)
                        nc.gpsimd.dma_start(
                            out=out_ext[rows, s0 * DIM : (s0 + cl) * DIM],
                            in_=g[:, :, :],
                        )
                    s0 += cl
    nc.finalize()
    return nc


_cache: dict = {}


def _pack_meta(tokens_i32: np.ndarray, dyn_i32: np.ndarray) -> np.ndarray:
    meta = np.empty((B, MW), dtype=np.float32)
    meta[:, 0:SEQ] = tokens_i32
    meta[:, SEQ] = dyn_i32
    meta[:, SEQ + 1 :] = np.arange(SEQ, dtype=np.float32)[None, :]
    return meta


def kernel(**inputs) -> np.ndarray:
    _ensure_profiling_hooks()
    tokens = np.asarray(inputs["tokenized_text"]).astype(np.int32)
    dyn = np.asarray(inputs["dynamic_bools"]).astype(np.int32)
    emb = np.asarray(inputs["token_embedding"], dtype=np.float32)
    da = np.asarray(inputs["da_vectors"], dtype=np.float32)
    ca = np.asarray(inputs["ca_vectors"], dtype=np.float32)
    table = np.ascontiguousarray(np.concatenate([emb, da, ca], axis=0))
    if TABLE_DT == "f16":
        table = table.astype(np.float16)
    meta = _pack_meta(tokens, dyn)

    if "nc" not in _cache:
        _cache["nc"] = _build()
    nc = _cache["nc"]

    in_maps = []
    for i in range(NCORES):
        rows = slice(i * BPC, (i + 1) * BPC)
        in_maps.append({"meta": meta[rows], "table": table})
    res = run_bass_kernel_spmd(nc, in_maps, core_ids=list(range(NCORES)))
    _cache["last_results"] = res
    out = np.concatenate(
        [
            res.results[i]["out"].reshape(BPC, SEQ, DIM).astype(np.float32)
            for i in range(NCORES)
        ],
        axis=0,
    )
    return out



# revision 9
# speedup vs baseline: 1.5270x; 1.0053x over previous
"""Trainium2 Bass kernel for nn_ContextAddition (ragged sequence insertion).

Math: for each row b with first-EOT position e = argmin{p: tok[b,p]==EOT} and
shift = 16 if dynamic_bools[b] else 8, the reference output reduces to a pure
row-gather from an extended embedding table T = [token_embedding; da; ca]:

    out[b,p] = T[ tok[b,p] ]            if p <  e
             = T[ VOCAB + (p - e) ]     if e <= p < e + shift   (da rows then ca rows)
             = T[ tok[b, p - shift] ]   if p >= e + shift

(The da insertion applies to all rows; the ca insertion only to dynamic rows,
and since da precedes ca in T, VOCAB + (p - e) indexes both uniformly.)

So the kernel computes an int32 index map [B,77] on-device with vector ops,
then row-gathers from an f16 table in DRAM via per-position indirect DMAs
(1536 B/row), staged through SBUF, written densely to an f16 output
(upcast to f32 host-side — lossless, the table is f16). Pure data parallel
over 8 cores (256 batch rows each); the table is replicated.

Device-input layout: tokens/dynamic_bools/position-iota are packed into one
f32 "meta" array [B, 2*SEQ+1] host-side so the whole per-tile index
computation hangs off a single input DMA (all values < 2^24, f32-exact).

PERF MODEL (measured, trn2 via axon, profiled core 0):
- Per core traffic: 30.28 MB gather read (19,712 rows x 1536 B) +
  30.28 MB f16 write = 60.6 MB. DMA-queue busy ~ 177 us/queue (16 queues,
  ~23.6 GB/s/queue busy rate => ~378 GB/s aggregate, at HBM peak).
- GpSimd (Q7/SWDGE) is the BOTTLENECK: each [128,1]-offset INDIRECT1D costs
  1103 ns exec + 309 ns inter-instruction gap (fixed Q7 launch overhead;
  measured identical in raw Block mode with zero sem waits, so it is NOT
  tile-framework overhead). 154 instructions => ~217 us serialized, + ~8 us
  ramp + ~8 us Q7 drain + ~9 us NEFF overhead => ~241 us exec. This is the
  architectural floor for per-position indirect gathers.
- Dead ends verified on HW (do not retry): multi-index indirect_dma_start
  ([128,cl] offsets) is broken in lowering both with contiguous dst
  (segments merge -> descriptor/index count mismatch) and padded dst (only
  partition 0's segments emitted, stride-cl index consumption).
  dma_gather (InstDMAGatherAnt) pairs cleanly (element i read from
  partition 16+(i%16), col i//16) but int16 indices address at most 32,768
  rows < 40,016 used rows, and the no-middle-skip semantics mean any
  range-split/dummy-row scheme doubles read traffic. Transpose-mode
  dma_gather would allow -1 placeholders but gathers a full dummy row for
  them (same traffic) and lands data transposed (needs PE/DVE transpose
  back). num_swdge_queues>1 is rejected by this build and dma_memcopy
  ucode is frozen to Q7 pair 0 anyway.
"""

import sys

import numpy as np

from concourse import bacc, bass, mybir
import concourse.tile as tile
from concourse.bass_utils import run_bass_kernel_spmd


def _ensure_profiling_hooks():
    """Make NTFF tracing under axon non-fatal / functional if BASS_TRACE is
    set by the caller: register the antenv.axon_hooks shim when the real
    module is absent, and make artifact upload failures non-fatal."""
    try:
        import antenv.axon_hooks  # noqa: F401
    except ImportError:
        try:
            import contextlib as _cl
            import types as _t

            import antenv
            from trn_agent_boot.trn_boot import _ntff_profile_via_ctypes

            hook = _ntff_profile_via_ctypes("/opt/axon/libaxon_pjrt.so")

            if hook is not None:
                _raw = hook

                @_cl.contextmanager
                def _safe(output_dir, device_ids):
                    # transient axon profiler failures (e.g. stop rc=-1)
                    # degrade to "no trace" instead of crashing the run
                    try:
                        cm = _raw(output_dir, device_ids)
                        cm.__enter__()
                    except Exception:
                        yield
                        return
                    try:
                        yield
                    finally:
                        try:
                            cm.__exit__(None, None, None)
                        except Exception:
                            pass

                hook = _safe

            mod = _t.ModuleType("antenv.axon_hooks")
            mod._hook = hook
            mod.set_axon_ntff_profile_hook = lambda h: setattr(mod, "_hook", h)
            mod.get_axon_ntff_profile_hook = lambda: mod._hook
            sys.modules["antenv.axon_hooks"] = mod
            antenv.axon_hooks = mod
        except Exception:
            pass
    from concourse import bass_utils as _bu

    if not getattr(_bu.upload_artifacts, "_safe_wrapped", False):
        _orig = _bu.upload_artifacts

        def _safe_upload(tmpdir):
            try:
                return _orig(tmpdir)
            except Exception:
                return f"file://{tmpdir}"

        _safe_upload._safe_wrapped = True
        _bu.upload_artifacts = _safe_upload

B, SEQ, DIM = 2048, 77, 768
VOCAB, EOT = 49408, 49407
INS = 16                       # appended rows: 8 da + 8 ca
TBL = VOCAB + INS
NCORES = 8
BPC = B // NCORES              # 256 batch rows per core
P = 128
NT = BPC // P                  # 2 partition tiles per core
SC = 11                        # seq chunk: 77 = 7 * 11
NCH = SEQ // SC
MW = 2 * SEQ + 1               # meta width: [tokens | dyn | pos]
TABLE_DT = "f16"               # "f32": exact; "f16": half-size table (rel err ~2e-4)
OUT_DT = "f16"                 # "f16": write output f16, upcast host-side (lossless
                               # vs the f16 table); halves write-side HBM traffic
GP_BUFS = 8                    # gather pool depth
DMA_SCRATCH = 16384            # SWDGE descriptor-ring carveout bytes
# With f16 HWDGE writes (no SWDGE cast-writes in the ring anymore) the
# old 6/5 interleave rationale is gone; bigger chunks = fewer, larger
# writes and a deeper gather pipeline.
CHUNKS = [11] * 7
# NOTE (verified on HW twice): multi-index indirect_dma_start ([128,cl]
# offsets) is broken in the walrus lowering — with a contiguous dst the
# per-partition segments merge into one descriptor (count mismatch vs
# offsets), and with a padded (non-mergeable) dst only partition 0's
# segments are emitted with stride-cl index consumption. Only the
# [128,1]-offset per-position form pairs correctly. The per-instruction
# cost (1103ns exec + 309ns issue gap) x 154 = 217us is the GpSimd
# serialization floor of this kernel.

f32 = mybir.dt.float32
i32 = mybir.dt.int32
Alu = mybir.AluOpType


def _build() -> bass.Bass:
    global TABLE_DT, OUT_DT, GP_BUFS, CHUNKS, DMA_SCRATCH
    chunks = CHUNKS if CHUNKS is not None else [SC] * NCH
    assert sum(chunks) == SEQ
    tdt = f32 if TABLE_DT == "f32" else mybir.dt.float16
    odt = f32 if OUT_DT == "f32" else mybir.dt.float16
    nc = bacc.Bacc("TRN2", dynamic_dma_scratch_size=DMA_SCRATCH)
    meta_ext = nc.declare_dram_parameter("meta", [BPC, MW], f32, isOutput=False)
    table_ext = nc.declare_dram_parameter("table", [TBL, DIM], tdt, isOutput=False)
    out_ext = nc.declare_dram_parameter("out", [BPC, SEQ * DIM], odt, isOutput=True)

    with tile.TileContext(nc) as tc:
        with (
            tc.tile_pool(name="small", bufs=2) as sp,
            tc.tile_pool(name="gath", bufs=GP_BUFS) as gp,
            tc.tile_pool(name="cast", bufs=3) as hp,
        ):
            for t in range(NT):
                rows = slice(t * P, (t + 1) * P)

                meta = sp.tile([P, MW], f32, tag="meta")
                nc.sync.dma_start(out=meta[:], in_=meta_ext[rows, :])
                tok = meta[:, 0:SEQ]
                dyn = meta[:, SEQ : SEQ + 1]
                pos = meta[:, SEQ + 1 : SEQ + 1 + SEQ]

                # e[b] = sum_p p * (tok == EOT)  (exactly one EOT per row)
                iseq = sp.tile([P, SEQ], f32, tag="iseq")
                nc.vector.tensor_scalar(
                    out=iseq[:], in0=tok, scalar1=float(EOT), scalar2=None,
                    op0=Alu.is_equal,
                )
                pe = sp.tile([P, SEQ], f32, tag="pe")
                nc.vector.tensor_tensor(out=pe[:], in0=iseq[:], in1=pos, op=Alu.mult)
                e = sp.tile([P, 1], f32, tag="e")
                nc.vector.tensor_reduce(
                    out=e[:], in_=pe[:], axis=mybir.AxisListType.X, op=Alu.add
                )

                # eth[b] = e + 8 + 8*dyn
                sh = sp.tile([P, 1], f32, tag="sh")
                nc.vector.tensor_scalar(
                    out=sh[:], in0=dyn, scalar1=8.0, scalar2=8.0,
                    op0=Alu.mult, op1=Alu.add,
                )
                eth = sp.tile([P, 1], f32, tag="eth")
                nc.vector.tensor_tensor(out=eth[:], in0=sh[:], in1=e[:], op=Alu.add)

                # mid = pos - e + VOCAB   (index into the da/ca rows)
                mid = sp.tile([P, SEQ], f32, tag="mid")
                nc.vector.tensor_scalar(
                    out=mid[:], in0=pos, scalar1=e[:], scalar2=float(VOCAB),
                    op0=Alu.subtract, op1=Alu.add,
                )

                # masks must be integer-typed for CopyPredicated on HW
                m1 = sp.tile([P, SEQ], i32, tag="m1")   # p < e
                nc.vector.tensor_scalar(
                    out=m1[:], in0=pos, scalar1=e[:], scalar2=None, op0=Alu.is_lt
                )
                m2 = sp.tile([P, SEQ], i32, tag="m2")   # p < e + shift
                nc.vector.tensor_scalar(
                    out=m2[:], in0=pos, scalar1=eth[:], scalar2=None, op0=Alu.is_lt
                )

                # tok shifted right by 8 and by 16 (cols < shift never selected)
                tm8 = sp.tile([P, SEQ], f32, tag="tm8")
                nc.vector.tensor_copy(out=tm8[:, 8:SEQ], in_=meta[:, 0 : SEQ - 8])
                nc.vector.tensor_copy(out=tm8[:, 0:8], in_=meta[:, 0:8])
                tm16 = sp.tile([P, SEQ], f32, tag="tm16")
                nc.vector.tensor_copy(out=tm16[:, 16:SEQ], in_=meta[:, 0 : SEQ - 16])
                nc.vector.tensor_copy(out=tm16[:, 0:16], in_=meta[:, 0:16])

                # sel = tm8 + dyn * (tm16 - tm8); overlay mid, then pre-EOT tokens
                dd = sp.tile([P, SEQ], f32, tag="dd")
                nc.vector.tensor_tensor(out=dd[:], in0=tm16[:], in1=tm8[:], op=Alu.subtract)
                ddm = sp.tile([P, SEQ], f32, tag="ddm")
                nc.vector.tensor_scalar(
                    out=ddm[:], in0=dd[:], scalar1=dyn, scalar2=None, op0=Alu.mult
                )
                sel = sp.tile([P, SEQ], f32, tag="sel")
                nc.vector.tensor_tensor(out=sel[:], in0=tm8[:], in1=ddm[:], op=Alu.add)
                nc.vector.copy_predicated(out=sel[:], mask=m2[:], data=mid[:])
                nc.vector.copy_predicated(out=sel[:], mask=m1[:], data=tok)

                idx = sp.tile([P, SEQ], i32, tag="idx")
                nc.vector.tensor_copy(out=idx[:], in_=sel[:])

                s0 = 0
                for c, cl in enumerate(chunks):
                    # one indirect DMA per position, [128,1] offsets (one
                    # index per partition): the HW DGE emits one descriptor
                    # per partition, consuming exactly one offset element
                    # each (multi-index-per-partition forms misbehave on HW)
                    g = gp.tile([P, cl, DIM], tdt, tag="g")
                    for j in range(cl):
                        nc.gpsimd.indirect_dma_start(
                            out=g[:, j, :],
                            out_offset=None,
                            in_=table_ext[:],
                            in_offset=bass.IndirectOffsetOnAxis(
                                ap=idx[:, s0 + j : s0 + j + 1], axis=0
                            ),
                        )
                    if TABLE_DT == OUT_DT:
                        # same dtype: plain HWDGE write, no cast anywhere
                        nc.sync.dma_start(
                            out=out_ext[rows, s0 * DIM : (s0 + cl) * DIM],
                            in_=g[:, :, :],
                        )
                    else:
                        # dtype cast during DMA requires SWDGE (gpsimd)
                        nc.gpsimd.dma_start(
                            out=out_ext[rows, s0 * DIM : (s0 + cl) * DIM],
                            in_=g[:, :, :],
                        )
                    s0 += cl
    nc.finalize()
    return nc


_cache: dict = {}


def _pack_meta(tokens_i32: np.ndarray, dyn_i32: np.ndarray) -> np.ndarray:
    meta = np.empty((B, MW), dtype=np.float32)
    meta[:, 0:SEQ] = tokens_i32
    meta[:, SEQ] = dyn_i32
    meta[:, SEQ + 1 :] = np.arange(SEQ, dtype=np.float32)[None, :]
    return meta


def kernel(**inputs) -> np.ndarray:
    _ensure_profiling_hooks()
    tokens = np.asarray(inputs["tokenized_text"]).astype(np.int32)
    dyn = np.asarray(inputs["dynamic_bools"]).astype(np.int32)
    emb = np.asarray(inputs["token_embedding"], dtype=np.float32)
    da = np.asarray(inputs["da_vectors"], dtype=np.float32)
    ca = np.asarray(inputs["ca_vectors"], dtype=np.float32)
    table = np.ascontiguousarray(np.concatenate([emb, da, ca], axis=0))
    if TABLE_DT == "f16":
        table = table.astype(np.float16)
    meta = _pack_meta(tokens, dyn)

    if "nc" not in _cache:
        _cache["nc"] = _build()
    nc = _cache["nc"]

    in_maps = []
    for i in range(NCORES):
        rows = slice(i * BPC, (i + 1) * BPC)
        in_maps.append({"meta": meta[rows], "table": table})
    res = run_bass_kernel_spmd(nc, in_maps, core_ids=list(range(NCORES)))
    _cache["last_results"] = res
    out = np.concatenate(
        [
            res.results[i]["out"].reshape(BPC, SEQ, DIM).astype(np.float32)
            for i in range(NCORES)
        ],
        axis=0,
    )
    return out

